# revision 31
# baseline (speedup 1.0000x reference)
"""Trainium2 Bass kernel for nn_DiagonalVariational.

out[i, d] = m[d] + sqrt(log_diag_L[d]^2 + 1e-6) * eps[i, d]

Sharding: data-parallel over the **d axis** — each of the 8 cores gets a
[2048, 2048] column slice of eps/out plus the matching [2048] slices of
m and log_diag_L. Column sharding (instead of n_sample sharding) makes
the per-core [d]-vector broadcast 8x smaller, small enough to do with a
stride-0 DMA read from DRAM (~2 MB extra HBM traffic, ~6 us) instead of
gpsimd partition_broadcast (which measures ~3x slower than its cost
model on HW and dominated n_sample-sharded variants).

Per-core kernel: partition = sample row, free = local d, 16 slabs of
[128, 2048] (1 MB DMAs). scale = sqrt(l^2 + jitter) (one Newton step —
the ACT Sqrt table is only ~1e-6 relative) is computed in a [128, 16]
view and staged through a DRAM scratch so the broadcast can re-read it
row-wise. Loads ride the SP HWDGE ring, stores the ACT ring, so stores
never head-of-line block the eps load stream. Each tile takes two fp32
tensor_tensor ops (mul scale_b, add m_b) on the vector engine; the tail
slab is split into quarter-width pieces so the kernel doesn't end on a
full-width compute+store chain.
"""

import sys

sys.path.insert(0, "/opt/trn_rl_repo")

import numpy as np

D = 16384
N_SAMPLE = 2048
N_CORES = 8
D_LOCAL = D // N_CORES  # 2048
P = 128
JITTER = 1e-6

_CACHE = {}


def _build(
    eps_bufs=8,
    slab_pair=1,
    gpsimd_slabs=0,
    tail_split=4,
    bcast_ring="sync",
    repeat=1,
    setup_in_loop=False,
):
    import contextlib

    import concourse.bacc as bacc
    import concourse.mybir as mybir
    from concourse.tile import TileContext

    DL = D_LOCAL
    n_groups = N_SAMPLE // (P * slab_pair)

    nc = bacc.Bacc("TRN2", target_bir_lowering=False, debug=False, num_devices=N_CORES)

    m_d = nc.dram_tensor("m", (DL,), mybir.dt.float32, kind="ExternalInput").ap()
    l_d = nc.dram_tensor(
        "log_diag_L", (DL,), mybir.dt.float32, kind="ExternalInput"
    ).ap()
    eps_d = nc.dram_tensor(
        "eps", (N_SAMPLE, DL), mybir.dt.float32, kind="ExternalInput"
    ).ap()
    out_d = nc.dram_tensor(
        "out", (N_SAMPLE, DL), mybir.dt.float32, kind="ExternalOutput"
    ).ap()

    with TileContext(nc) as tc:
        with (
            tc.tile_pool(name="setup", bufs=1) as setup_pool,
            tc.tile_pool(name="dram", bufs=1, space="DRAM") as dram_pool,
            tc.tile_pool(name="eps", bufs=eps_bufs) as eps_pool,
        ):
            W = DL // P
            l_t = setup_pool.tile([P, W], mybir.dt.float32)
            sq_t = setup_pool.tile([P, W], mybir.dt.float32)
            scale_t = setup_pool.tile([P, W], mybir.dt.float32)
            rcp_t = setup_pool.tile([P, W], mybir.dt.float32)
            scratch = dram_pool.tile([P, W], mybir.dt.float32)
            scratch_flat = scratch[:].rearrange("a b -> (a b)")
            s_b = setup_pool.tile([P, DL], mybir.dt.float32)
            m_b = setup_pool.tile([P, DL], mybir.dt.float32)

            bcast_eng = {
                "gpsimd": nc.gpsimd,
                "scalar": nc.scalar,
                "sync": nc.sync,
            }[bcast_ring]

            def setup():
                bcast_eng.dma_start(
                    out=m_b[:], in_=m_d[None, :].to_broadcast((P, DL))
                )
                nc.sync.dma_start(
                    out=l_t[:], in_=l_d.rearrange("(a b) -> a b", b=W)
                )
                nc.vector.tensor_mul(out=sq_t[:], in0=l_t[:], in1=l_t[:])
                nc.vector.tensor_scalar_add(out=sq_t[:], in0=sq_t[:], scalar1=JITTER)
                nc.scalar.activation(
                    scale_t[:], sq_t[:], mybir.ActivationFunctionType.Sqrt
                )
                nc.vector.reciprocal(out=rcp_t[:], in_=scale_t[:])
                nc.vector.tensor_mul(out=rcp_t[:], in0=rcp_t[:], in1=sq_t[:])
                nc.vector.tensor_add(out=scale_t[:], in0=scale_t[:], in1=rcp_t[:])
                nc.vector.tensor_scalar_mul(
                    out=scale_t[:], in0=scale_t[:], scalar1=0.5
                )
                nc.scalar.dma_start(out=scratch[:], in_=scale_t[:])
                bcast_eng.dma_start(
                    out=s_b[:], in_=scratch_flat[None, :].to_broadcast((P, DL))
                )

            if not setup_in_loop:
                setup()

            loop_ctx = (
                tc.For_i(0, repeat, 1) if repeat > 1 else contextlib.nullcontext()
            )
            with loop_ctx:
                if setup_in_loop:
                    setup()
                gp_set = set(range(1, 1 + gpsimd_slabs))
                for g in range(n_groups):
                    rs = slice(g * P * slab_pair, (g + 1) * P * slab_pair)
                    src = eps_d[rs, :].rearrange("(s p) d -> p s d", p=P)
                    dst = out_d[rs, :].rearrange("(s p) d -> p s d", p=P)
                    t = eps_pool.tile([P, slab_pair, DL], mybir.dt.float32, tag="t")
                    eng = nc.gpsimd if g in gp_set else nc.vector
                    last = g == n_groups - 1
                    strips = tail_split if (last and tail_split > 1) else 1
                    step = DL // strips
                    for j in range(0, DL, step):
                        js = slice(j, j + step)
                        # 3D tensor ops: in1 broadcasts along the middle
                        # (slab) axis with stride 0
                        sv = s_b[:, None, js].to_broadcast((P, slab_pair, step))
                        mv = m_b[:, None, js].to_broadcast((P, slab_pair, step))
                        nc.sync.dma_start(out=t[:, :, js], in_=src[:, :, js])
                        eng.tensor_mul(out=t[:, :, js], in0=t[:, :, js], in1=sv)
                        eng.tensor_add(out=t[:, :, js], in0=t[:, :, js], in1=mv)
                        nc.scalar.dma_start(out=dst[:, :, js], in_=t[:, :, js])

    nc.compile()
    return nc


def _get_nc():
    if "nc" not in _CACHE:
        _CACHE["nc"] = _build()
    return _CACHE["nc"]


def _shard_inputs(m, log_diag_L, eps):
    m = np.ascontiguousarray(m, dtype=np.float32)
    log_diag_L = np.ascontiguousarray(log_diag_L, dtype=np.float32)
    eps = np.ascontiguousarray(eps, dtype=np.float32)
    return [
        {
            "m": m[i * D_LOCAL : (i + 1) * D_LOCAL],
            "log_diag_L": log_diag_L[i * D_LOCAL : (i + 1) * D_LOCAL],
            "eps": np.ascontiguousarray(eps[:, i * D_LOCAL : (i + 1) * D_LOCAL]),
        }
        for i in range(N_CORES)
    ]


def _gather_out(shards):
    return np.concatenate(list(shards), axis=1)


def kernel(m, log_diag_L, eps, **run_kwargs):
    from concourse import bass_utils

    nc = _get_nc()
    in_maps = _shard_inputs(m, log_diag_L, eps)
    res = bass_utils.run_bass_kernel_spmd(
        nc, in_maps, core_ids=list(range(N_CORES)), **run_kwargs
    )
    out = _gather_out(r["out"] for r in res.results)
    if run_kwargs:
        _CACHE["last_results"] = res
    return out
